# revision 1
# baseline (speedup 1.0000x reference)
"""Trainium2 Bass kernel for 2-layer RGCN (mean aggregation) on 8 NeuronCores.

Design:
  - dst-sharded: core k owns destination rows [k*6250, (k+1)*6250), padded to
    6272 = 49 tiles of 128 rows. Each core computes its output rows entirely,
    so no cross-core reduction is needed.
  - Edges are grouped on the host by (core, relation, dst_tile); each group is
    padded to 3 chunks of 128 edge slots (max observed occupancy ~334 < 384).
  - Per chunk: indirect-DMA gather of the 128 source rows (bf16), a one-hot
    mask [128e, 128d] built on DVE from iota/is_equal, and a TensorE matmul
    mask^T @ msgs accumulated in PSUM -> per-(r, tile) segment sums.
  - Mean normalization (1/cnt per (r, dst)) is folded into the PSUM->SBUF copy
    as a per-partition scale on the Scalar engine.
  - The per-relation transform (agg @ W_r summed over r) runs on TensorE using
    PE-transposed aggregates as the stationary operand; root term and bias are
    extra accumulating matmuls into the same PSUM tile.
  - Two launches: L1 produces h shards; the host concatenates them (pure data
    movement) and launch 2 consumes the full h for its gathers. This avoids
    on-device collectives entirely.
  - All matmul operands are bf16 (fp32 PSUM accumulation); validated end-to-end
    numerically at ~0.3% relative error vs the fp32 reference.
"""
import numpy as np
import ml_dtypes

N = 50000
E = 800000
R = 8
IN, HID, OUT = 512, 256, 512
NCORES = 8
SHARD = 6250
TILES = 52                 # padded tile count (49 real + 3 empty)
LTILES = 13                # tiles per launch (4 launches per layer)
PSH = TILES * 128          # 6656 padded rows per core
C = 3                      # chunks per (relation, dst-tile) group
NCH = R * TILES * C        # chunks per core
bf16 = ml_dtypes.bfloat16

_pending_trace = {"l1": None, "l2": None}


# ---------------------------------------------------------------------------
# Workarounds for this container's walrus build (single sync-wait per
# instruction) and missing NTFF profile hook under axon.
# ---------------------------------------------------------------------------
def _install_tilefix():
    import concourse.mybir as mybir
    import concourse.tile as tile_mod
    from concourse.vector_clock import ScopedClock

    if getattr(tile_mod.TileContext, "_rgcn_patched", False):
        return
    counter = [0]

    def split_multiwaits(nc):
        for f in nc.m.functions:
            for bb in f.blocks:
                out = []
                changed = False
                for inst in bb.instructions:
                    si = inst.sync_info
                    waits = list(si.on_wait) if si is not None else []
                    if len(waits) > 1:
                        changed = True
                        for w in waits[:-1]:
                            counter[0] += 1
                            nop = mybir.InstNoOp(
                                name=f"I-wsplit-{counter[0]}", ins=[], outs=[])
                            nop.engine = inst.engine
                            nop.sync_info = mybir.SyncInfo(
                                on_wait=[w], on_update=[])
                            nc.register_instruction(nop, overwrite=True)
                            out.append(nop)
                        si.on_wait = waits[-1:]
                    out.append(inst)
                if changed:
                    bb.instructions = out

    def patched_drain_and_barrier(self, tick_clock, wait_clock):
        nc = self.nc
        drain_inst = nc.sync.drain()
        wait_clock.add_sem_waits(
            drain_inst.ins, ScopedClock({None: tick_clock.global_clock}))
        nc.all_engine_barrier()
        assert self.sems is not None
        popped = nc._tile_sem_poison_stack.pop()
        assert popped is self._sem_poison
        nc.clear_and_free_semaphores(list(self.sems.allocated().values()))
        nc.all_engine_barrier()
        split_multiwaits(nc)

    tile_mod.TileContext._drain_and_barrier = patched_drain_and_barrier
    tile_mod.TileContext._rgcn_patched = True


def _install_ntff_hook():
    import sys, types
    if 'antenv.axon_hooks' in sys.modules:
        return
    try:
        try:
            from trn_agent_boot.trn_boot import _ntff_profile_via_ctypes
        except ImportError:
            sys.path.insert(0, '/root/.axon_site')
            from trn_agent_boot.trn_boot import _ntff_profile_via_ctypes
        hook = _ntff_profile_via_ctypes('/opt/axon/libaxon_pjrt.so')
    except Exception:
        return
    mod = types.ModuleType('antenv.axon_hooks')
    mod.get_axon_ntff_profile_hook = lambda: hook
    mod.set_axon_ntff_profile_hook = lambda h: None
    sys.modules['antenv.axon_hooks'] = mod


# ---------------------------------------------------------------------------
# Host preprocessing: edge grouping/padding + per-core index/mask/scale arrays
# ---------------------------------------------------------------------------
def _host_prep(src, dst, et):
    src = src.astype(np.int64)
    dst = dst.astype(np.int64)
    et = et.astype(np.int64)

    seg = et * N + dst
    cnt = np.bincount(seg, minlength=R * N).astype(np.float32)
    inv = np.where(cnt > 0, 1.0 / np.maximum(cnt, 1), 0.0).astype(np.float32)

    core_of = dst // SHARD
    dloc = dst - core_of * SHARD
    tile_of = dloc // 128
    dst_in_tile = (dloc % 128).astype(np.float32)

    pad_src = (src // SHARD) * PSH + (src % SHARD)  # index into padded h

    per_core = []
    for c in range(NCORES):
        eids = np.nonzero(core_of == c)[0]
        key = et[eids] * TILES + tile_of[eids]
        order = np.argsort(key, kind='stable')
        eids = eids[order]
        key = key[order]
        starts = np.searchsorted(key, np.arange(R * TILES))
        ends = np.searchsorted(key, np.arange(R * TILES) + 1)
        ns = ends - starts
        if ns.max() > C * 128:
            raise RuntimeError(f"group overflow: {ns.max()} > {C*128}")

        slot_src = np.zeros((R * TILES, C * 128), np.int32)
        slot_src2 = np.zeros((R * TILES, C * 128), np.int32)
        slot_dl = np.full((R * TILES, C * 128), -1.0, np.float32)
        for g in range(R * TILES):
            n = ns[g]
            e = eids[starts[g]:ends[g]]
            slot_src[g, :n] = src[e]
            slot_src2[g, :n] = pad_src[e]
            slot_dl[g, :n] = dst_in_tile[e]

        # chunk ch of group g -> column g*C + ch; slot j in chunk -> partition j
        def to_cols(a, dt):
            return np.ascontiguousarray(
                a.reshape(R * TILES * C, 128).T).astype(dt)

        inv_cols = np.zeros((128, R * TILES), np.float32)
        base = c * SHARD
        rows = base + (np.arange(TILES * 128) % (TILES * 128))
        for t in range(TILES):
            rr = base + t * 128 + np.arange(128)
            ok = rr < (c + 1) * SHARD
            rc = np.minimum(rr, N - 1)
            for r in range(R):
                inv_cols[:, r * TILES + t] = np.where(ok, inv[r * N + rc], 0.0)

        per_core.append(dict(
            idx1=to_cols(slot_src, np.int32),
            idx2=to_cols(slot_src2, np.int32),
            dstloc=to_cols(slot_dl, np.float32),
            invc=np.ascontiguousarray(inv_cols),
        ))
    return per_core


def _pack_weights(W, nchunk):
    # W [R, K, M] with K = nchunk*128 -> [128, R*nchunk*M], block (r, c) at
    # columns (r*nchunk + c)*M
    Rr, K, M = W.shape
    out = np.zeros((128, Rr * nchunk * M), bf16)
    for r in range(Rr):
        for c in range(nchunk):
            out[:, (r * nchunk + c) * M:(r * nchunk + c + 1) * M] = \
                W[r, c * 128:(c + 1) * 128, :].astype(bf16)
    return out


def _pack_single(Wm, nchunk):
    K, M = Wm.shape
    out = np.zeros((128, nchunk * M), bf16)
    for c in range(nchunk):
        out[:, c * M:(c + 1) * M] = Wm[c * 128:(c + 1) * 128, :].astype(bf16)
    return out


def _shard_T(xf, c, width, t0, base_stride=SHARD):
    # rows of core c for launch tiles [t0, t0+LTILES), transposed. The source
    # holds core c's rows at offset c*base_stride with SHARD valid rows.
    nch = width // 128
    base = c * base_stride
    lo = base + t0 * 128
    hi = min(base + SHARD, lo + LTILES * 128)
    nrows = max(0, hi - lo)
    blk = np.zeros((width, LTILES * 128), np.float32)
    if nrows > 0:
        blk[:, :nrows] = xf[lo:hi].T
    out = np.zeros((128, nch * LTILES * 128), bf16)
    W = LTILES * 128
    for cc in range(nch):
        out[:, cc * W:(cc + 1) * W] = blk[cc * 128:(cc + 1) * 128].astype(bf16)
    return out


def _slice_cols(a, t0, per_tile):
    # a [128, R*TILES*per_tile] grouped (r, tile) -> launch cols for tiles
    # [t0, t0+LTILES) of every relation, relaid out as (r, local_tile)
    cols = []
    for r in range(R):
        s = (r * TILES + t0) * per_tile
        cols.append(a[:, s:s + LTILES * per_tile])
    return np.ascontiguousarray(np.concatenate(cols, axis=1))


# ---------------------------------------------------------------------------
# Device kernel builders
# ---------------------------------------------------------------------------
def _build_layer(layer):
    import concourse.bass as bass
    import concourse.mybir as mybir
    from concourse.tile import TileContext

    F = IN if layer == 1 else HID        # message width
    H = HID if layer == 1 else OUT       # output width
    FC = F // 128                        # feature chunks (4 or 2)
    NSRC = N if layer == 1 else NCORES * PSH
    T = LTILES
    LNCH = R * T * C

    nc = bass.Bass()
    xsrc = nc.dram_tensor('xsrc', [NSRC, F], mybir.dt.bfloat16, kind='ExternalInput')
    xT = nc.dram_tensor('xT', [128, FC * T * 128], mybir.dt.bfloat16, kind='ExternalInput')
    Wsb = nc.dram_tensor('Wsb', [128, R * FC * H], mybir.dt.bfloat16, kind='ExternalInput')
    rootsb = nc.dram_tensor('rootsb', [128, FC * H], mybir.dt.bfloat16, kind='ExternalInput')
    brow = nc.dram_tensor('brow', [1, H], mybir.dt.bfloat16, kind='ExternalInput')
    idx = nc.dram_tensor('idx', [128, LNCH], mybir.dt.int32, kind='ExternalInput')
    dstloc = nc.dram_tensor('dstloc', [128, LNCH], mybir.dt.float32, kind='ExternalInput')
    invc = nc.dram_tensor('invc', [128, R * T], mybir.dt.float32, kind='ExternalInput')
    iota = nc.dram_tensor('iota', [128, 128], mybir.dt.bfloat16, kind='ExternalInput')
    ident = nc.dram_tensor('ident', [128, 128], mybir.dt.bfloat16, kind='ExternalInput')
    out_dt = mybir.dt.bfloat16 if layer == 1 else mybir.dt.float32
    yout = nc.dram_tensor('yout', [T * 128, H], out_dt, kind='ExternalOutput')

    with TileContext(nc) as tc:
        with tc.tile_pool(name='const', bufs=1) as cp, \
             tc.tile_pool(name='gather', bufs=6) as gp, \
             tc.tile_pool(name='masks', bufs=6) as mp, \
             tc.tile_pool(name='aggs', bufs=3) as ap_, \
             tc.tile_pool(name='aggts', bufs=3) as atp, \
             tc.tile_pool(name='hout', bufs=3) as hp, \
             tc.tile_pool(name='pagg', bufs=2, space='PSUM') as pagg, \
             tc.tile_pool(name='ptr', bufs=2, space='PSUM') as ptr, \
             tc.tile_pool(name='pout', bufs=2, space='PSUM') as pout:

            xT_sb = cp.tile([128, FC * T * 128], mybir.dt.bfloat16)
            nc.sync.dma_start(out=xT_sb[:], in_=xT[:])
            W_sb = cp.tile([128, R * FC * H], mybir.dt.bfloat16)
            nc.sync.dma_start(out=W_sb[:], in_=Wsb[:])
            root_sb = cp.tile([128, FC * H], mybir.dt.bfloat16)
            nc.sync.dma_start(out=root_sb[:], in_=rootsb[:])
            b_sb = cp.tile([1, H], mybir.dt.bfloat16)
            nc.sync.dma_start(out=b_sb[:], in_=brow[:])
            ones_sb = cp.tile([1, 128], mybir.dt.bfloat16)
            nc.vector.memset(ones_sb[:], 1.0)
            idx_sb = cp.tile([128, LNCH], mybir.dt.int32)
            nc.sync.dma_start(out=idx_sb[:], in_=idx[:])
            dl_sb = cp.tile([128, LNCH], mybir.dt.float32)
            nc.sync.dma_start(out=dl_sb[:], in_=dstloc[:])
            inv_sb = cp.tile([128, R * T], mybir.dt.float32)
            nc.sync.dma_start(out=inv_sb[:], in_=invc[:])
            iota_sb = cp.tile([128, 128], mybir.dt.bfloat16)
            nc.sync.dma_start(out=iota_sb[:], in_=iota[:])
            id_sb = cp.tile([128, 128], mybir.dt.bfloat16)
            nc.sync.dma_start(out=id_sb[:], in_=ident[:])

            for t in range(T):
                opsum = pout.tile([128, H], mybir.dt.float32)
                for r in range(R):
                    g = r * T + t
                    agg = pagg.tile([128, F], mybir.dt.float32)
                    for ch in range(C):
                        col = g * C + ch
                        msgs = gp.tile([128, F], mybir.dt.bfloat16, tag='msgs')
                        nc.gpsimd.indirect_dma_start(
                            out=msgs[:], out_offset=None, in_=xsrc[:],
                            in_offset=bass.IndirectOffsetOnAxis(
                                ap=idx_sb[:, col:col + 1], axis=0))
                        mask = mp.tile([128, 128], mybir.dt.bfloat16, tag='mask')
                        nc.vector.tensor_scalar(
                            out=mask[:], in0=iota_sb[:],
                            scalar1=dl_sb[:, col:col + 1], scalar2=None,
                            op0=mybir.AluOpType.is_equal)
                        nc.tensor.matmul(
                            out=agg[:], lhsT=mask[:], rhs=msgs[:],
                            start=(ch == 0), stop=(ch == C - 1))
                    # scale by 1/cnt (per dst row) while copying PSUM->SBUF
                    agg_s = ap_.tile([128, F], mybir.dt.bfloat16, tag='aggs')
                    nc.scalar.activation(
                        out=agg_s[:], in_=agg[:],
                        func=mybir.ActivationFunctionType.Copy,
                        scale=inv_sb[:, g:g + 1])
                    # transpose agg_s -> aggT (feature-major) via TensorE
                    trp = ptr.tile([128, F], mybir.dt.bfloat16, tag='trp')
                    for c2 in range(FC):
                        nc.tensor.transpose(
                            out=trp[:, c2 * 128:(c2 + 1) * 128],
                            in_=agg_s[:, c2 * 128:(c2 + 1) * 128],
                            identity=id_sb[:])
                    aggT = atp.tile([128, F], mybir.dt.bfloat16, tag='aggT')
                    nc.vector.tensor_copy(out=aggT[:], in_=trp[:])
                    # transform: opsum += agg @ W_r
                    for c2 in range(FC):
                        nc.tensor.matmul(
                            out=opsum[:],
                            lhsT=aggT[:, c2 * 128:(c2 + 1) * 128],
                            rhs=W_sb[:, (r * FC + c2) * H:(r * FC + c2 + 1) * H],
                            start=(r == 0 and c2 == 0), stop=False)
                # root term + bias
                for c2 in range(FC):
                    nc.tensor.matmul(
                        out=opsum[:],
                        lhsT=xT_sb[:, (c2 * T + t) * 128:(c2 * T + t + 1) * 128],
                        rhs=root_sb[:, c2 * H:(c2 + 1) * H],
                        start=False, stop=False)
                nc.tensor.matmul(
                    out=opsum[:], lhsT=ones_sb[:], rhs=b_sb[:],
                    start=False, stop=True)

                if layer == 1:
                    h_t = hp.tile([128, H], mybir.dt.bfloat16, tag='ht')
                    nc.scalar.activation(
                        out=h_t[:], in_=opsum[:],
                        func=mybir.ActivationFunctionType.Relu)
                    nc.sync.dma_start(
                        out=yout[t * 128:(t + 1) * 128, :], in_=h_t[:])
                else:
                    nrm2 = hp.tile([128, 1], mybir.dt.float32, tag='n2')
                    sq = hp.tile([128, OUT], mybir.dt.float32, tag='sq')
                    nc.scalar.activation(
                        out=sq[:], in_=opsum[:],
                        func=mybir.ActivationFunctionType.Square,
                        accum_out=nrm2[:])
                    srt = hp.tile([128, 1], mybir.dt.float32, tag='srt')
                    nc.scalar.activation(
                        out=srt[:], in_=nrm2[:],
                        func=mybir.ActivationFunctionType.Sqrt)
                    nc.vector.tensor_scalar_max(srt[:], srt[:], 1e-12)
                    rcp = hp.tile([128, 1], mybir.dt.float32, tag='rcp')
                    nc.vector.reciprocal(rcp[:], srt[:])
                    o_t = hp.tile([128, OUT], mybir.dt.float32, tag='ot')
                    nc.scalar.activation(
                        out=o_t[:], in_=opsum[:],
                        func=mybir.ActivationFunctionType.Copy,
                        scale=rcp[:])
                    nc.sync.dma_start(
                        out=yout[t * 128:(t + 1) * 128, :], in_=o_t[:])
    return nc


def _run(nc, in_maps, trace=False):
    from concourse import bass_utils
    res = bass_utils.run_bass_kernel_spmd(
        nc, in_maps, core_ids=list(range(NCORES)), trace=trace)
    return res


# ---------------------------------------------------------------------------
# Entry point
# ---------------------------------------------------------------------------
def kernel(x, W1, root1, b1, W2, root2, b2, src, dst, edge_type,
           _trace=None):
    global C, NCH
    _install_tilefix()
    _install_ntff_hook()

    # size chunk capacity to the actual densest (relation, dst-tile) group
    _d = np.asarray(dst).astype(np.int64)
    _e = np.asarray(edge_type).astype(np.int64)
    _g = ((_d // SHARD) * R + _e) * TILES + (_d % SHARD) // 128
    _mx = int(np.bincount(_g, minlength=NCORES * R * TILES).max())
    C = max(3, -(-_mx // 128))
    NCH = R * TILES * C

    x = np.asarray(x, np.float32)
    per_core = _host_prep(np.asarray(src), np.asarray(dst),
                          np.asarray(edge_type))

    iota_np = np.broadcast_to(np.arange(128, dtype=np.float32),
                              (128, 128)).astype(bf16)
    ident_np = np.eye(128, dtype=np.float32).astype(bf16)

    x_bf = x.astype(bf16)
    W1p = _pack_weights(np.asarray(W1, np.float32), IN // 128)
    r1p = _pack_single(np.asarray(root1, np.float32), IN // 128)
    b1p = np.asarray(b1, np.float32)[None, :].astype(bf16)
    W2p = _pack_weights(np.asarray(W2, np.float32), HID // 128)
    r2p = _pack_single(np.asarray(root2, np.float32), HID // 128)
    b2p = np.asarray(b2, np.float32)[None, :].astype(bf16)

    # ---- layer 1: 4 launches of LTILES tiles each ----
    nc1 = _build_layer(1)
    h_full = np.zeros((NCORES * PSH, HID), bf16)
    t_l1 = 0
    for li in range(TILES // LTILES):
        t0 = li * LTILES
        in_maps1 = []
        for c in range(NCORES):
            pc = per_core[c]
            in_maps1.append(dict(
                xsrc=x_bf, xT=_shard_T(x, c, IN, t0), Wsb=W1p, rootsb=r1p,
                brow=b1p, idx=_slice_cols(pc['idx1'], t0, C),
                dstloc=_slice_cols(pc['dstloc'], t0, C),
                invc=_slice_cols(pc['invc'], t0, 1),
                iota=iota_np, ident=ident_np))
        res1 = _run(nc1, in_maps1, trace=(_trace == f'l1_{li}'))
        if res1.exec_time_ns:
            t_l1 += res1.exec_time_ns
        for c in range(NCORES):
            h_full[c * PSH + t0 * 128: c * PSH + (t0 + LTILES) * 128] = \
                res1.results[c]['yout']
    _pending_trace['l1'] = t_l1 or None

    # ---- layer 2: 4 launches ----
    h_f32 = h_full.astype(np.float32)
    nc2 = _build_layer(2)
    out = np.empty((N, OUT), np.float32)
    t_l2 = 0
    for li in range(TILES // LTILES):
        t0 = li * LTILES
        in_maps2 = []
        for c in range(NCORES):
            pc = per_core[c]
            in_maps2.append(dict(
                xsrc=h_full, xT=_shard_T(h_f32, c, HID, t0, PSH), Wsb=W2p,
                rootsb=r2p, brow=b2p, idx=_slice_cols(pc['idx2'], t0, C),
                dstloc=_slice_cols(pc['dstloc'], t0, C),
                invc=_slice_cols(pc['invc'], t0, 1),
                iota=iota_np, ident=ident_np))
        res2 = _run(nc2, in_maps2, trace=(_trace == f'l2_{li}'))
        if res2.exec_time_ns:
            t_l2 += res2.exec_time_ns
        for c in range(NCORES):
            rows0 = c * SHARD + t0 * 128
            nrows = max(0, min((c + 1) * SHARD, rows0 + LTILES * 128) - rows0)
            if nrows > 0:
                out[rows0:rows0 + nrows] = \
                    res2.results[c]['yout'][:nrows].astype(np.float32)
    _pending_trace['l2'] = t_l2 or None
    return out



# revision 12
# speedup vs baseline: 1.2849x; 1.2849x over previous
"""Trainium2 Bass kernel for 2-layer RGCN (mean aggregation) on 8 NeuronCores.

v2 design (vs v1 baseline):
  - dst-sharded: core k owns destination rows [k*6250, (k+1)*6250) = 49 tiles
    of 128 rows. Each core computes its output rows entirely; no collectives.
  - ONE launch per layer covering all 49 tiles (v1: 4 launches of 13).
  - Gathers are batched: per dst-tile, TWO dma_gather (SWDGE ucode)
    instructions fetch all ~24 chunks of source rows (v1: 24 indirect_dma
    instructions at ~1us fixed Pool-engine cost each -> was the bottleneck).
    dma_gather uses int16 indices, so the source table is split at row 32768
    into an A table (rows < 32768) and a B table; per (relation, tile) group
    the slots are packed [A-edges | B-edges] into ceil(nA/128) A-chunks and
    ceil(nB/128) B-chunks (typ. 2 + 1).
  - Aggregation per (r, tile): one-hot mask matmuls (mask built on DVE from
    iota/is_equal) accumulate chunk contributions in PSUM; mean normalization
    (1/cnt) is a per-partition scale folded into the PSUM->SBUF copy on the
    Scalar engine; PE transposes feed the per-relation transform matmuls,
    with the root term and bias as extra accumulating matmuls.
  - idxs for dma_gather are replicated into all eight 16-partition blocks
    (one per GPSIMD Q7 core).
  - All matmul operands bf16 (fp32 PSUM accumulation); ~0.3% rel err.
"""
import numpy as np
import ml_dtypes

N = 50000
E = 800000
R = 8
IN, HID, OUT = 512, 256, 512
NCORES = 8
SHARD = 6250
TILES = 49                 # ceil(6250/128)
PSH = TILES * 128          # 6272 rows per core in padded h layout
SPLIT = 32768              # A/B source-table split for int16 dma_gather idxs
bf16 = ml_dtypes.bfloat16

_pending_trace = {"l1": None, "l2": None}


# ---------------------------------------------------------------------------
# Workarounds for this container's walrus build (single sync-wait per
# instruction) and missing NTFF profile hook under axon.
# ---------------------------------------------------------------------------
def _install_tilefix():
    import concourse.mybir as mybir
    import concourse.tile as tile_mod
    from concourse.vector_clock import ScopedClock

    if getattr(tile_mod.TileContext, "_rgcn_patched", False):
        return
    counter = [0]

    def split_multiwaits(nc):
        for f in nc.m.functions:
            for bb in f.blocks:
                out = []
                changed = False
                for inst in bb.instructions:
                    si = inst.sync_info
                    waits = list(si.on_wait) if si is not None else []
                    if len(waits) > 1:
                        changed = True
                        for w in waits[:-1]:
                            counter[0] += 1
                            nop = mybir.InstNoOp(
                                name=f"I-wsplit-{counter[0]}", ins=[], outs=[])
                            nop.engine = inst.engine
                            nop.sync_info = mybir.SyncInfo(
                                on_wait=[w], on_update=[])
                            nc.register_instruction(nop, overwrite=True)
                            out.append(nop)
                        si.on_wait = waits[-1:]
                    out.append(inst)
                if changed:
                    bb.instructions = out

    def patched_drain_and_barrier(self, tick_clock, wait_clock):
        nc = self.nc
        drain_inst = nc.sync.drain()
        wait_clock.add_sem_waits(
            drain_inst.ins, ScopedClock({None: tick_clock.global_clock}))
        nc.all_engine_barrier()
        assert self.sems is not None
        popped = nc._tile_sem_poison_stack.pop()
        assert popped is self._sem_poison
        nc.clear_and_free_semaphores(list(self.sems.allocated().values()))
        nc.all_engine_barrier()
        split_multiwaits(nc)

    tile_mod.TileContext._drain_and_barrier = patched_drain_and_barrier
    tile_mod.TileContext._rgcn_patched = True


def _install_ntff_hook():
    import sys, types
    if 'antenv.axon_hooks' in sys.modules:
        return
    try:
        try:
            from trn_agent_boot.trn_boot import _ntff_profile_via_ctypes
        except ImportError:
            sys.path.insert(0, '/root/.axon_site')
            from trn_agent_boot.trn_boot import _ntff_profile_via_ctypes
        hook = _ntff_profile_via_ctypes('/opt/axon/libaxon_pjrt.so')
    except Exception:
        return
    mod = types.ModuleType('antenv.axon_hooks')
    mod.get_axon_ntff_profile_hook = lambda: hook
    mod.set_axon_ntff_profile_hook = lambda h: None
    sys.modules['antenv.axon_hooks'] = mod


# ---------------------------------------------------------------------------
# Host preprocessing
# ---------------------------------------------------------------------------
def _wrap16(idxs):
    """dma_gather idx layout: idx j at (partition 16k + j%16, free j//16),
    replicated into all eight Q7 16-partition blocks."""
    n = len(idxs)
    arr = np.zeros((128, max(1, n // 16)), np.int16)
    j = np.arange(n)
    for k in range(8):
        arr[16 * k + j % 16, j // 16] = idxs
    return arr


def _host_prep(src, dst, et, pad_src):
    """Group edges per (core, relation, dst-tile), split by src-half, and
    compute the uniform chunk schedule.  Returns (sched_a, sched_b, per_core).
    pad_src: per-edge row index into the padded source layout (shared by both
    layers: x is staged into the same [core][tile][row] layout as h)."""
    seg = et * N + dst
    cnt = np.bincount(seg, minlength=R * N).astype(np.float32)
    inv = np.where(cnt > 0, 1.0 / np.maximum(cnt, 1), 0.0).astype(np.float32)

    core_of = dst // SHARD
    dloc = dst - core_of * SHARD
    tile_of = dloc // 128
    dst_in_tile = (dloc % 128).astype(np.float32)

    # per (core, r, t): A/B edge id lists
    groups = {}
    for c in range(NCORES):
        eids = np.nonzero(core_of == c)[0]
        key = (et[eids] * TILES + tile_of[eids])
        order = np.argsort(key, kind='stable')
        eids = eids[order]
        key = key[order]
        starts = np.searchsorted(key, np.arange(R * TILES))
        ends = np.searchsorted(key, np.arange(R * TILES) + 1)
        groups[c] = (eids, starts, ends)

    # uniform schedule: per (r, t) chunk counts, max over cores
    sched_a = np.zeros((R, TILES), np.int64)
    sched_b = np.zeros((R, TILES), np.int64)
    for c in range(NCORES):
        eids, starts, ends = groups[c]
        for r in range(R):
            for t in range(TILES):
                g = r * TILES + t
                e = eids[starts[g]:ends[g]]
                na = int((pad_src[e] < SPLIT).sum())
                nb = len(e) - na
                sched_a[r, t] = max(sched_a[r, t], -(-na // 128))
                sched_b[r, t] = max(sched_b[r, t], -(-nb // 128))
    sched_a = np.maximum(sched_a, 1)
    sched_b = np.maximum(sched_b, 1)

    per_core = []
    for c in range(NCORES):
        eids, starts, ends = groups[c]
        # chunk stream per tile: [A-chunks r0..r7 | B-chunks r0..r7]
        slot_ix = []   # per-slot source row (A/B-table relative)
        slot_dl = []   # dst row-in-tile per slot (-1 pad)
        tile_na = []   # per tile: number of A chunks
        tile_nb = []
        for t in range(TILES):
            dls, s1 = [], []
            for half in (0, 1):
                for r in range(R):
                    g = r * TILES + t
                    e = eids[starts[g]:ends[g]]
                    ha = pad_src[e] < SPLIT
                    e = e[ha] if half == 0 else e[~ha]
                    nch = int(sched_a[r, t] if half == 0 else sched_b[r, t])
                    ns = nch * 128
                    v1 = np.zeros(ns, np.int64)
                    dl = np.full(ns, -1.0, np.float32)
                    v1[:len(e)] = pad_src[e] - (0 if half == 0 else SPLIT)
                    dl[:len(e)] = dst_in_tile[e]
                    s1.append(v1)
                    dls.append(dl)
            na = int(sched_a[:, t].sum())
            nb = int(sched_b[:, t].sum())
            tile_na.append(na)
            tile_nb.append(nb)
            slot_ix.append(np.concatenate(s1))
            slot_dl.append(np.concatenate(dls))

        # dma_gather idx arrays per tile, concatenated along free dim
        def idx_tensor(slots, na_list, which):
            cols = []
            for t in range(TILES):
                na = na_list[t] * 128
                sl = slots[t]
                part = sl[:na] if which == 0 else sl[na:]
                cols.append(_wrap16(part))
            return np.ascontiguousarray(np.concatenate(cols, axis=1))

        dl_cols = np.concatenate(slot_dl).reshape(-1, 128).T
        dl_cols = np.ascontiguousarray(dl_cols.astype(np.float32))

        inv_cols = np.zeros((128, R * TILES), np.float32)
        base = c * SHARD
        for t in range(TILES):
            rr = base + t * 128 + np.arange(128)
            ok = rr < (c + 1) * SHARD
            rc = np.minimum(rr, N - 1)
            for r in range(R):
                inv_cols[:, r * TILES + t] = np.where(ok, inv[r * N + rc], 0.0)

        per_core.append(dict(
            idxa=idx_tensor(slot_ix, tile_na, 0),
            idxb=idx_tensor(slot_ix, tile_na, 1),
            dstloc=dl_cols,
            invc=np.ascontiguousarray(inv_cols),
        ))
    return sched_a, sched_b, per_core


def _pack_weights(W, nchunk):
    Rr, K, M = W.shape
    out = np.zeros((128, Rr * nchunk * M), bf16)
    for r in range(Rr):
        for ch in range(nchunk):
            out[:, (r * nchunk + ch) * M:(r * nchunk + ch + 1) * M] = \
                W[r, ch * 128:(ch + 1) * 128, :].astype(bf16)
    return out


def _pack_single(Wm, nchunk):
    K, M = Wm.shape
    out = np.zeros((128, nchunk * M), bf16)
    for ch in range(nchunk):
        out[:, ch * M:(ch + 1) * M] = Wm[ch * 128:(ch + 1) * 128, :].astype(bf16)
    return out


def _shard_T(xf, c, width, base_stride=SHARD):
    """Core c's 6250 rows transposed into [128, FC*TILES*128] bf16 blocks:
    block (fc, t) holds x[rows of tile t, fc*128:(fc+1)*128].T"""
    nch = width // 128
    base = c * base_stride
    nrows = min(SHARD, TILES * 128)
    blk = np.zeros((width, TILES * 128), np.float32)
    blk[:, :nrows] = xf[base:base + nrows].T
    out = np.zeros((128, nch * TILES * 128), bf16)
    W_ = TILES * 128
    for ch in range(nch):
        out[:, ch * W_:(ch + 1) * W_] = blk[ch * 128:(ch + 1) * 128].astype(bf16)
    return out


# ---------------------------------------------------------------------------
# Device kernel builder
# ---------------------------------------------------------------------------
def _build_layer(layer, sched_a, sched_b):
    import concourse.bass as bass
    import concourse.mybir as mybir
    from concourse.tile import TileContext
    from concourse import library_config

    F = IN if layer == 1 else HID        # message width
    H = HID if layer == 1 else OUT       # output width
    FC = F // 128
    NSRC = NCORES * PSH                  # padded source layout (both layers)
    tile_na = [int(sched_a[:, t].sum()) for t in range(TILES)]
    tile_nb = [int(sched_b[:, t].sum()) for t in range(TILES)]
    TOTC = sum(tile_na) + sum(tile_nb)
    TOT_A16 = sum(na * 8 for na in tile_na)   # int16 idx cols (128 slots -> 8)
    TOT_B16 = sum(nb * 8 for nb in tile_nb)
    CTMAX = max(tile_na[t] + tile_nb[t] for t in range(TILES))

    nc = bass.Bass()
    xsrc = nc.dram_tensor('xsrc', [NSRC, F], mybir.dt.bfloat16, kind='ExternalInput')
    xT = nc.dram_tensor('xT', [128, FC * TILES * 128], mybir.dt.bfloat16, kind='ExternalInput')
    Wsb = nc.dram_tensor('Wsb', [128, R * FC * H], mybir.dt.bfloat16, kind='ExternalInput')
    rootsb = nc.dram_tensor('rootsb', [128, FC * H], mybir.dt.bfloat16, kind='ExternalInput')
    brow = nc.dram_tensor('brow', [1, H], mybir.dt.bfloat16, kind='ExternalInput')
    idxa = nc.dram_tensor('idxa', [128, TOT_A16], mybir.dt.int16, kind='ExternalInput')
    idxb = nc.dram_tensor('idxb', [128, TOT_B16], mybir.dt.int16, kind='ExternalInput')
    dstloc = nc.dram_tensor('dstloc', [128, TOTC], mybir.dt.float32, kind='ExternalInput')
    invc = nc.dram_tensor('invc', [128, R * TILES], mybir.dt.float32, kind='ExternalInput')
    iota = nc.dram_tensor('iota', [128, 128], mybir.dt.bfloat16, kind='ExternalInput')
    ident = nc.dram_tensor('ident', [128, 128], mybir.dt.bfloat16, kind='ExternalInput')
    out_dt = mybir.dt.bfloat16 if layer == 1 else mybir.dt.float32
    yout = nc.dram_tensor('yout', [TILES * 128, H], out_dt, kind='ExternalOutput')

    with TileContext(nc) as tc:
        with tc.tile_pool(name='const', bufs=1) as cp, \
             tc.tile_pool(name='gather', bufs=2) as gp, \
             tc.tile_pool(name='xtp', bufs=2) as xp, \
             tc.tile_pool(name='masks', bufs=8) as mp, \
             tc.tile_pool(name='aggs', bufs=3) as ap_, \
             tc.tile_pool(name='aggts', bufs=3) as atp, \
             tc.tile_pool(name='hout', bufs=3) as hp, \
             tc.tile_pool(name='pagg', bufs=2, space='PSUM') as pagg, \
             tc.tile_pool(name='ptr', bufs=2, space='PSUM') as ptr, \
             tc.tile_pool(name='pout', bufs=2, space='PSUM') as pout:

            nc.gpsimd.load_library(library_config.mlp)

            W_sb = cp.tile([128, R * FC * H], mybir.dt.bfloat16)
            nc.sync.dma_start(out=W_sb[:], in_=Wsb[:])
            root_sb = cp.tile([128, FC * H], mybir.dt.bfloat16)
            nc.sync.dma_start(out=root_sb[:], in_=rootsb[:])
            b_sb = cp.tile([1, H], mybir.dt.bfloat16)
            nc.sync.dma_start(out=b_sb[:], in_=brow[:])
            ones_sb = cp.tile([1, 128], mybir.dt.bfloat16)
            nc.vector.memset(ones_sb[:], 1.0)
            ia_sb = cp.tile([128, TOT_A16], mybir.dt.int16)
            nc.sync.dma_start(out=ia_sb[:], in_=idxa[:])
            ib_sb = cp.tile([128, TOT_B16], mybir.dt.int16)
            nc.sync.dma_start(out=ib_sb[:], in_=idxb[:])
            dl_sb = cp.tile([128, TOTC], mybir.dt.float32)
            nc.sync.dma_start(out=dl_sb[:], in_=dstloc[:])
            inv_sb = cp.tile([128, R * TILES], mybir.dt.float32)
            nc.sync.dma_start(out=inv_sb[:], in_=invc[:])
            iota_sb = cp.tile([128, 128], mybir.dt.bfloat16)
            nc.sync.dma_start(out=iota_sb[:], in_=iota[:])
            id_sb = cp.tile([128, 128], mybir.dt.bfloat16)
            nc.sync.dma_start(out=id_sb[:], in_=ident[:])

            # one register per distinct num_idxs value, reused across tiles
            reg_cache = {}

            def nreg(v):
                if v not in reg_cache:
                    reg_cache[v] = nc.gpsimd.to_reg(v)
                return reg_cache[v]

            a16_off = 0
            b16_off = 0
            col_off = 0
            for t in range(TILES):
                na, nb = tile_na[t], tile_nb[t]
                ct = na + nb
                msgs = gp.tile([128, CTMAX, F], mybir.dt.bfloat16, tag='msgs')
                nc.gpsimd.dma_gather(
                    msgs[:, 0:na, :], xsrc[0:SPLIT, :],
                    ia_sb[:, a16_off:a16_off + na * 8],
                    na * 128, nreg(na * 128), F, single_packet=False)
                nc.gpsimd.dma_gather(
                    msgs[:, na:ct, :], xsrc[SPLIT:NSRC, :],
                    ib_sb[:, b16_off:b16_off + nb * 8],
                    nb * 128, nreg(nb * 128), F, single_packet=False)
                a16_off += na * 8
                b16_off += nb * 8

                xt_t = xp.tile([128, FC * 128], mybir.dt.bfloat16, tag='xt')
                for fc in range(FC):
                    nc.sync.dma_start(
                        out=xt_t[:, fc * 128:(fc + 1) * 128],
                        in_=xT[:, (fc * TILES + t) * 128:(fc * TILES + t + 1) * 128])

                # per (r, t): chunk positions in the stream
                # A-chunks: offset sum(sched_a[:r, t]); B: na + sum(sched_b[:r, t])
                a_pos = np.concatenate([[0], np.cumsum(sched_a[:, t])])
                b_pos = np.concatenate([[0], np.cumsum(sched_b[:, t])])

                opsum = pout.tile([128, H], mybir.dt.float32)
                for r in range(R):
                    chunks = [int(a_pos[r]) + i for i in range(int(sched_a[r, t]))] + \
                             [na + int(b_pos[r]) + i for i in range(int(sched_b[r, t]))]
                    agg = pagg.tile([128, F], mybir.dt.float32)
                    for ci, ch in enumerate(chunks):
                        col = col_off + ch
                        mask = mp.tile([128, 128], mybir.dt.bfloat16, tag='mask')
                        nc.vector.tensor_scalar(
                            out=mask[:], in0=iota_sb[:],
                            scalar1=dl_sb[:, col:col + 1], scalar2=None,
                            op0=mybir.AluOpType.is_equal)
                        nc.tensor.matmul(
                            out=agg[:], lhsT=mask[:], rhs=msgs[:, ch, :],
                            start=(ci == 0), stop=(ci == len(chunks) - 1))
                    g = r * TILES + t
                    agg_s = ap_.tile([128, F], mybir.dt.bfloat16, tag='aggs')
                    nc.scalar.activation(
                        out=agg_s[:], in_=agg[:],
                        func=mybir.ActivationFunctionType.Copy,
                        scale=inv_sb[:, g:g + 1])
                    trp = ptr.tile([128, F], mybir.dt.bfloat16, tag='trp')
                    for c2 in range(FC):
                        nc.tensor.transpose(
                            out=trp[:, c2 * 128:(c2 + 1) * 128],
                            in_=agg_s[:, c2 * 128:(c2 + 1) * 128],
                            identity=id_sb[:])
                    aggT = atp.tile([128, F], mybir.dt.bfloat16, tag='aggT')
                    nc.vector.tensor_copy(out=aggT[:], in_=trp[:])
                    for c2 in range(FC):
                        nc.tensor.matmul(
                            out=opsum[:],
                            lhsT=aggT[:, c2 * 128:(c2 + 1) * 128],
                            rhs=W_sb[:, (r * FC + c2) * H:(r * FC + c2 + 1) * H],
                            start=(r == 0 and c2 == 0), stop=False)
                col_off += ct
                # root term + bias
                for c2 in range(FC):
                    nc.tensor.matmul(
                        out=opsum[:],
                        lhsT=xt_t[:, c2 * 128:(c2 + 1) * 128],
                        rhs=root_sb[:, c2 * H:(c2 + 1) * H],
                        start=False, stop=False)
                nc.tensor.matmul(
                    out=opsum[:], lhsT=ones_sb[:], rhs=b_sb[:],
                    start=False, stop=True)

                if layer == 1:
                    h_t = hp.tile([128, H], mybir.dt.bfloat16, tag='ht')
                    nc.scalar.activation(
                        out=h_t[:], in_=opsum[:],
                        func=mybir.ActivationFunctionType.Relu)
                    nc.sync.dma_start(
                        out=yout[t * 128:(t + 1) * 128, :], in_=h_t[:])
                else:
                    nrm2 = hp.tile([128, 1], mybir.dt.float32, tag='n2')
                    sq = hp.tile([128, OUT], mybir.dt.float32, tag='sq')
                    nc.scalar.activation(
                        out=sq[:], in_=opsum[:],
                        func=mybir.ActivationFunctionType.Square,
                        accum_out=nrm2[:])
                    srt = hp.tile([128, 1], mybir.dt.float32, tag='srt')
                    nc.scalar.activation(
                        out=srt[:], in_=nrm2[:],
                        func=mybir.ActivationFunctionType.Sqrt)
                    nc.vector.tensor_scalar_max(srt[:], srt[:], 1e-12)
                    rcp = hp.tile([128, 1], mybir.dt.float32, tag='rcp')
                    nc.vector.reciprocal(rcp[:], srt[:])
                    o_t = hp.tile([128, OUT], mybir.dt.float32, tag='ot')
                    nc.scalar.activation(
                        out=o_t[:], in_=opsum[:],
                        func=mybir.ActivationFunctionType.Copy,
                        scale=rcp[:])
                    nc.sync.dma_start(
                        out=yout[t * 128:(t + 1) * 128, :], in_=o_t[:])

    import concourse.mybir as mybir2
    mybir2.codegen_inst_isa_subclasses(nc)
    return nc


def _run(nc, in_maps, trace=False):
    from concourse import bass_utils
    res = bass_utils.run_bass_kernel_spmd(
        nc, in_maps, core_ids=list(range(NCORES)), trace=trace)
    return res


# ---------------------------------------------------------------------------
# Entry point
# ---------------------------------------------------------------------------
def kernel(x, W1, root1, b1, W2, root2, b2, src, dst, edge_type,
           _trace=None):
    _install_tilefix()
    _install_ntff_hook()

    src = np.asarray(src).astype(np.int64)
    dst = np.asarray(dst).astype(np.int64)
    et = np.asarray(edge_type).astype(np.int64)
    x = np.asarray(x, np.float32)

    pad_src = (src // SHARD) * PSH + (src % SHARD)   # row in padded h layout
    sched_a, sched_b, per_core = _host_prep(src, dst, et, pad_src)

    iota_np = np.broadcast_to(np.arange(128, dtype=np.float32),
                              (128, 128)).astype(bf16)
    ident_np = np.eye(128, dtype=np.float32).astype(bf16)

    # stage x into the same padded [core][tile][row] layout as h
    x_pad = np.zeros((NCORES * PSH, IN), bf16)
    for c in range(NCORES):
        x_pad[c * PSH:c * PSH + SHARD] = x[c * SHARD:(c + 1) * SHARD].astype(bf16)
    W1p = _pack_weights(np.asarray(W1, np.float32), IN // 128)
    r1p = _pack_single(np.asarray(root1, np.float32), IN // 128)
    b1p = np.asarray(b1, np.float32)[None, :].astype(bf16)
    W2p = _pack_weights(np.asarray(W2, np.float32), HID // 128)
    r2p = _pack_single(np.asarray(root2, np.float32), HID // 128)
    b2p = np.asarray(b2, np.float32)[None, :].astype(bf16)

    # ---- layer 1: single launch ----
    nc1 = _build_layer(1, sched_a, sched_b)
    in_maps1 = []
    for c in range(NCORES):
        pc = per_core[c]
        in_maps1.append(dict(
            xsrc=x_pad, xT=_shard_T(x, c, IN), Wsb=W1p, rootsb=r1p,
            brow=b1p, idxa=pc['idxa'], idxb=pc['idxb'],
            dstloc=pc['dstloc'], invc=pc['invc'],
            iota=iota_np, ident=ident_np))
    res1 = _run(nc1, in_maps1, trace=(_trace == 'l1_0'))
    _pending_trace['l1'] = res1.exec_time_ns
    h_full = np.zeros((NCORES * PSH, HID), bf16)
    for c in range(NCORES):
        h_full[c * PSH:(c + 1) * PSH] = res1.results[c]['yout']

    # ---- layer 2: single launch ----
    h_f32 = h_full.astype(np.float32)
    nc2 = _build_layer(2, sched_a, sched_b)
    in_maps2 = []
    for c in range(NCORES):
        pc = per_core[c]
        in_maps2.append(dict(
            xsrc=h_full, xT=_shard_T(h_f32, c, HID, PSH), Wsb=W2p,
            rootsb=r2p, brow=b2p, idxa=pc['idxa'], idxb=pc['idxb'],
            dstloc=pc['dstloc'], invc=pc['invc'],
            iota=iota_np, ident=ident_np))
    res2 = _run(nc2, in_maps2, trace=(_trace == 'l2_0'))
    _pending_trace['l2'] = res2.exec_time_ns
    out = np.empty((N, OUT), np.float32)
    for c in range(NCORES):
        out[c * SHARD:(c + 1) * SHARD] = \
            res2.results[c]['yout'][:SHARD].astype(np.float32)
    return out


# revision 22
# speedup vs baseline: 1.5598x; 1.2139x over previous
"""Trainium2 Bass kernel for 2-layer RGCN (mean aggregation) on 8 NeuronCores.

v2 design (vs v1 baseline):
  - dst-sharded: core k owns destination rows [k*6250, (k+1)*6250) = 49 tiles
    of 128 rows. Each core computes its output rows entirely; no collectives.
  - ONE launch per layer covering all 49 tiles (v1: 4 launches of 13).
  - Gathers are batched: per dst-tile, TWO dma_gather (SWDGE ucode)
    instructions fetch all ~24 chunks of source rows (v1: 24 indirect_dma
    instructions at ~1us fixed Pool-engine cost each -> was the bottleneck).
    dma_gather uses int16 indices, so the source table is split at row 32768
    into an A table (rows < 32768) and a B table; per (relation, tile) group
    the slots are packed [A-edges | B-edges] into ceil(nA/128) A-chunks and
    ceil(nB/128) B-chunks (typ. 2 + 1).
  - Aggregation per (r, tile): one-hot mask matmuls (mask built on DVE from
    iota/is_equal) accumulate chunk contributions in PSUM; mean normalization
    (1/cnt) is a per-partition scale folded into the PSUM->SBUF copy on the
    Scalar engine; PE transposes feed the per-relation transform matmuls,
    with the root term and bias as extra accumulating matmuls.
  - idxs for dma_gather are replicated into all eight 16-partition blocks
    (one per GPSIMD Q7 core).
  - All matmul operands bf16 (fp32 PSUM accumulation); ~0.3% rel err.
"""
import numpy as np
import ml_dtypes

N = 50000
E = 800000
R = 8
IN, HID, OUT = 512, 256, 512
NCORES = 8
SHARD = 6250
TILES = 49                 # ceil(6250/128)
PSH = TILES * 128          # 6272 rows per core in padded h layout
SPLIT = 32768              # A/B source-table split for int16 dma_gather idxs
bf16 = ml_dtypes.bfloat16

_pending_trace = {"l1": None, "l2": None}


# ---------------------------------------------------------------------------
# Workarounds for this container's walrus build (single sync-wait per
# instruction) and missing NTFF profile hook under axon.
# ---------------------------------------------------------------------------
def _install_tilefix():
    import concourse.mybir as mybir
    import concourse.tile as tile_mod
    from concourse.vector_clock import ScopedClock

    if getattr(tile_mod.TileContext, "_rgcn_patched", False):
        return
    counter = [0]

    def split_multiwaits(nc):
        for f in nc.m.functions:
            for bb in f.blocks:
                out = []
                changed = False
                for inst in bb.instructions:
                    si = inst.sync_info
                    waits = list(si.on_wait) if si is not None else []
                    if len(waits) > 1:
                        changed = True
                        for w in waits[:-1]:
                            counter[0] += 1
                            nop = mybir.InstNoOp(
                                name=f"I-wsplit-{counter[0]}", ins=[], outs=[])
                            nop.engine = inst.engine
                            nop.sync_info = mybir.SyncInfo(
                                on_wait=[w], on_update=[])
                            nc.register_instruction(nop, overwrite=True)
                            out.append(nop)
                        si.on_wait = waits[-1:]
                    out.append(inst)
                if changed:
                    bb.instructions = out

    def patched_drain_and_barrier(self, tick_clock, wait_clock):
        nc = self.nc
        drain_inst = nc.sync.drain()
        wait_clock.add_sem_waits(
            drain_inst.ins, ScopedClock({None: tick_clock.global_clock}))
        nc.all_engine_barrier()
        assert self.sems is not None
        popped = nc._tile_sem_poison_stack.pop()
        assert popped is self._sem_poison
        nc.clear_and_free_semaphores(list(self.sems.allocated().values()))
        nc.all_engine_barrier()
        split_multiwaits(nc)

    tile_mod.TileContext._drain_and_barrier = patched_drain_and_barrier
    tile_mod.TileContext._rgcn_patched = True


def _install_ntff_hook():
    import sys, types
    if 'antenv.axon_hooks' in sys.modules:
        return
    try:
        try:
            from trn_agent_boot.trn_boot import _ntff_profile_via_ctypes
        except ImportError:
            sys.path.insert(0, '/root/.axon_site')
            from trn_agent_boot.trn_boot import _ntff_profile_via_ctypes
        hook = _ntff_profile_via_ctypes('/opt/axon/libaxon_pjrt.so')
    except Exception:
        return
    mod = types.ModuleType('antenv.axon_hooks')
    mod.get_axon_ntff_profile_hook = lambda: hook
    mod.set_axon_ntff_profile_hook = lambda h: None
    sys.modules['antenv.axon_hooks'] = mod


# ---------------------------------------------------------------------------
# Host preprocessing
# ---------------------------------------------------------------------------
def _wrap16(idxs):
    """dma_gather idx layout: idx j at (partition 16k + j%16, free j//16),
    replicated into all eight Q7 16-partition blocks."""
    n = len(idxs)
    arr = np.zeros((128, max(1, n // 16)), np.int16)
    j = np.arange(n)
    for k in range(8):
        arr[16 * k + j % 16, j // 16] = idxs
    return arr


def _host_prep(src, dst, et, pad_src):
    """Group edges per (core, relation, dst-tile), split by src-half, and
    compute the uniform chunk schedule.  Returns (sched_a, sched_b, per_core).
    pad_src: per-edge row index into the padded source layout (shared by both
    layers: x is staged into the same [core][tile][row] layout as h)."""
    seg = et * N + dst
    cnt = np.bincount(seg, minlength=R * N).astype(np.float32)
    inv = np.where(cnt > 0, 1.0 / np.maximum(cnt, 1), 0.0).astype(np.float32)

    core_of = dst // SHARD
    dloc = dst - core_of * SHARD
    tile_of = dloc // 128
    dst_in_tile = (dloc % 128).astype(np.float32)

    # per (core, r, t): A/B edge id lists
    groups = {}
    for c in range(NCORES):
        eids = np.nonzero(core_of == c)[0]
        key = (et[eids] * TILES + tile_of[eids])
        order = np.argsort(key, kind='stable')
        eids = eids[order]
        key = key[order]
        starts = np.searchsorted(key, np.arange(R * TILES))
        ends = np.searchsorted(key, np.arange(R * TILES) + 1)
        groups[c] = (eids, starts, ends)

    # uniform schedule: per (r, t) chunk counts, max over cores
    sched_a = np.zeros((R, TILES), np.int64)
    sched_b = np.zeros((R, TILES), np.int64)
    for c in range(NCORES):
        eids, starts, ends = groups[c]
        for r in range(R):
            for t in range(TILES):
                g = r * TILES + t
                e = eids[starts[g]:ends[g]]
                na = int((pad_src[e] < SPLIT).sum())
                nb = len(e) - na
                sched_a[r, t] = max(sched_a[r, t], -(-na // 128))
                sched_b[r, t] = max(sched_b[r, t], -(-nb // 128))
    sched_a = np.maximum(sched_a, 1)
    sched_b = np.maximum(sched_b, 1)

    per_core = []
    for c in range(NCORES):
        eids, starts, ends = groups[c]
        # chunk stream per tile: [A-chunks r0..r7 | B-chunks r0..r7]
        slot_ix = []   # per-slot source row (A/B-table relative)
        slot_dl = []   # dst row-in-tile per slot (-1 pad)
        tile_na = []   # per tile: number of A chunks
        tile_nb = []
        for t in range(TILES):
            dls, s1 = [], []
            for half in (0, 1):
                for r in range(R):
                    g = r * TILES + t
                    e = eids[starts[g]:ends[g]]
                    ha = pad_src[e] < SPLIT
                    e = e[ha] if half == 0 else e[~ha]
                    nch = int(sched_a[r, t] if half == 0 else sched_b[r, t])
                    ns = nch * 128
                    v1 = np.zeros(ns, np.int64)
                    dl = np.full(ns, -1.0, np.float32)
                    v1[:len(e)] = pad_src[e] - (0 if half == 0 else SPLIT)
                    dl[:len(e)] = dst_in_tile[e]
                    s1.append(v1)
                    dls.append(dl)
            na = int(sched_a[:, t].sum())
            nb = int(sched_b[:, t].sum())
            tile_na.append(na)
            tile_nb.append(nb)
            slot_ix.append(np.concatenate(s1))
            slot_dl.append(np.concatenate(dls))

        # dma_gather idx arrays per tile, concatenated along free dim
        def idx_tensor(slots, na_list, which):
            cols = []
            for t in range(TILES):
                na = na_list[t] * 128
                sl = slots[t]
                part = sl[:na] if which == 0 else sl[na:]
                cols.append(_wrap16(part))
            return np.ascontiguousarray(np.concatenate(cols, axis=1))

        dl_cols = np.concatenate(slot_dl).reshape(-1, 128).T
        dl_cols = np.ascontiguousarray(dl_cols.astype(np.float32))

        # host-built one-hot masks [128, TOTC*128] bf16: column block ch holds
        # mask[p, d] = (dstloc[p, ch] == d)
        dl_i = dl_cols.astype(np.int32)           # [128, TOTC], -1 pad
        eye = np.arange(128, dtype=np.int32)
        masks = (dl_i[:, :, None] == eye[None, None, :]).astype(bf16)
        masks = np.ascontiguousarray(masks.reshape(128, -1))

        inv_cols = np.zeros((128, R * TILES), np.float32)
        base = c * SHARD
        for t in range(TILES):
            rr = base + t * 128 + np.arange(128)
            ok = rr < (c + 1) * SHARD
            rc = np.minimum(rr, N - 1)
            for r in range(R):
                inv_cols[:, r * TILES + t] = np.where(ok, inv[r * N + rc], 0.0)

        per_core.append(dict(
            idxa=idx_tensor(slot_ix, tile_na, 0),
            idxb=idx_tensor(slot_ix, tile_na, 1),
            masks=masks,
            invc=np.ascontiguousarray(inv_cols),
        ))
    return sched_a, sched_b, per_core


def _pack_weights(W, nchunk):
    Rr, K, M = W.shape
    out = np.zeros((128, Rr * nchunk * M), bf16)
    for r in range(Rr):
        for ch in range(nchunk):
            out[:, (r * nchunk + ch) * M:(r * nchunk + ch + 1) * M] = \
                W[r, ch * 128:(ch + 1) * 128, :].astype(bf16)
    return out


def _pack_single(Wm, nchunk):
    K, M = Wm.shape
    out = np.zeros((128, nchunk * M), bf16)
    for ch in range(nchunk):
        out[:, ch * M:(ch + 1) * M] = Wm[ch * 128:(ch + 1) * 128, :].astype(bf16)
    return out


def _shard_T(xf, c, width, base_stride=SHARD):
    """Core c's 6250 rows transposed into [128, FC*TILES*128] bf16 blocks:
    block (fc, t) holds x[rows of tile t, fc*128:(fc+1)*128].T"""
    nch = width // 128
    base = c * base_stride
    nrows = min(SHARD, TILES * 128)
    blk = np.zeros((width, TILES * 128), np.float32)
    blk[:, :nrows] = xf[base:base + nrows].T
    out = np.zeros((128, nch * TILES * 128), bf16)
    W_ = TILES * 128
    for ch in range(nch):
        out[:, ch * W_:(ch + 1) * W_] = blk[ch * 128:(ch + 1) * 128].astype(bf16)
    return out


# ---------------------------------------------------------------------------
# Device kernel builder
# ---------------------------------------------------------------------------
def _build_layer(layer, sched_a, sched_b):
    import concourse.bass as bass
    import concourse.mybir as mybir
    from concourse.tile import TileContext
    from concourse import library_config

    F = IN if layer == 1 else HID        # message width
    H = HID if layer == 1 else OUT       # output width
    FC = F // 128
    NSRC = NCORES * PSH                  # padded source layout (both layers)
    tile_na = [int(sched_a[:, t].sum()) for t in range(TILES)]
    tile_nb = [int(sched_b[:, t].sum()) for t in range(TILES)]
    TOTC = sum(tile_na) + sum(tile_nb)
    TOT_A16 = sum(na * 8 for na in tile_na)   # int16 idx cols (128 slots -> 8)
    TOT_B16 = sum(nb * 8 for nb in tile_nb)
    CTMAX = max(tile_na[t] + tile_nb[t] for t in range(TILES))

    nc = bass.Bass()
    xsrc = nc.dram_tensor('xsrc', [NSRC, F], mybir.dt.bfloat16, kind='ExternalInput')
    xT = nc.dram_tensor('xT', [128, FC * TILES * 128], mybir.dt.bfloat16, kind='ExternalInput')
    Wsb = nc.dram_tensor('Wsb', [128, R * FC * H], mybir.dt.bfloat16, kind='ExternalInput')
    rootsb = nc.dram_tensor('rootsb', [128, FC * H], mybir.dt.bfloat16, kind='ExternalInput')
    brow = nc.dram_tensor('brow', [1, H], mybir.dt.bfloat16, kind='ExternalInput')
    idxa = nc.dram_tensor('idxa', [128, TOT_A16], mybir.dt.int16, kind='ExternalInput')
    idxb = nc.dram_tensor('idxb', [128, TOT_B16], mybir.dt.int16, kind='ExternalInput')
    masks = nc.dram_tensor('masks', [128, TOTC * 128], mybir.dt.bfloat16, kind='ExternalInput')
    invc = nc.dram_tensor('invc', [128, R * TILES], mybir.dt.float32, kind='ExternalInput')
    ident = nc.dram_tensor('ident', [128, 128], mybir.dt.bfloat16, kind='ExternalInput')
    out_dt = mybir.dt.bfloat16 if layer == 1 else mybir.dt.float32
    yout = nc.dram_tensor('yout', [TILES * 128, H], out_dt, kind='ExternalOutput')

    with TileContext(nc) as tc:
        with tc.tile_pool(name='const', bufs=1) as cp, \
             tc.tile_pool(name='gather', bufs=3) as gp, \
             tc.tile_pool(name='xtp', bufs=2) as xp, \
             tc.tile_pool(name='masks', bufs=3) as mp, \
             tc.tile_pool(name='aggs', bufs=3) as ap_, \
             tc.tile_pool(name='aggts', bufs=3) as atp, \
             tc.tile_pool(name='hout', bufs=3) as hp, \
             tc.tile_pool(name='pagg', bufs=2, space='PSUM') as pagg, \
             tc.tile_pool(name='ptr', bufs=2, space='PSUM') as ptr, \
             tc.tile_pool(name='pout', bufs=2, space='PSUM') as pout:

            nc.gpsimd.load_library(library_config.mlp)

            W_sb = cp.tile([128, R * FC * H], mybir.dt.bfloat16)
            nc.sync.dma_start(out=W_sb[:], in_=Wsb[:])
            root_sb = cp.tile([128, FC * H], mybir.dt.bfloat16)
            nc.sync.dma_start(out=root_sb[:], in_=rootsb[:])
            b_sb = cp.tile([1, H], mybir.dt.bfloat16)
            nc.sync.dma_start(out=b_sb[:], in_=brow[:])
            ones_sb = cp.tile([1, 128], mybir.dt.bfloat16)
            nc.vector.memset(ones_sb[:], 1.0)
            ia_sb = cp.tile([128, TOT_A16], mybir.dt.int16)
            nc.sync.dma_start(out=ia_sb[:], in_=idxa[:])
            ib_sb = cp.tile([128, TOT_B16], mybir.dt.int16)
            nc.sync.dma_start(out=ib_sb[:], in_=idxb[:])
            inv_sb = cp.tile([128, R * TILES], mybir.dt.float32)
            nc.sync.dma_start(out=inv_sb[:], in_=invc[:])
            id_sb = cp.tile([128, 128], mybir.dt.bfloat16)
            nc.sync.dma_start(out=id_sb[:], in_=ident[:])

            # one register per distinct num_idxs value, reused across tiles
            reg_cache = {}

            def nreg(v):
                if v not in reg_cache:
                    reg_cache[v] = nc.gpsimd.to_reg(v)
                return reg_cache[v]

            a16_off = 0
            b16_off = 0
            col_off = 0
            for t in range(TILES):
                na, nb = tile_na[t], tile_nb[t]
                ct = na + nb
                msgs = gp.tile([128, CTMAX, F], mybir.dt.bfloat16, tag='msgs')
                nc.gpsimd.dma_gather(
                    msgs[:, 0:na, :], xsrc[0:SPLIT, :],
                    ia_sb[:, a16_off:a16_off + na * 8],
                    na * 128, nreg(na * 128), F, single_packet=False)
                nc.gpsimd.dma_gather(
                    msgs[:, na:ct, :], xsrc[SPLIT:NSRC, :],
                    ib_sb[:, b16_off:b16_off + nb * 8],
                    nb * 128, nreg(nb * 128), F, single_packet=False)
                a16_off += na * 8
                b16_off += nb * 8

                xt_t = xp.tile([128, FC * 128], mybir.dt.bfloat16, tag='xt')
                for fc in range(FC):
                    nc.sync.dma_start(
                        out=xt_t[:, fc * 128:(fc + 1) * 128],
                        in_=xT[:, (fc * TILES + t) * 128:(fc * TILES + t + 1) * 128])
                mk_t = mp.tile([128, CTMAX * 128], mybir.dt.bfloat16, tag='mk')
                nc.scalar.dma_start(
                    out=mk_t[:, 0:ct * 128],
                    in_=masks[:, col_off * 128:(col_off + ct) * 128])

                # per (r, t): chunk positions in the stream
                # A-chunks: offset sum(sched_a[:r, t]); B: na + sum(sched_b[:r, t])
                a_pos = np.concatenate([[0], np.cumsum(sched_a[:, t])])
                b_pos = np.concatenate([[0], np.cumsum(sched_b[:, t])])

                opsum = pout.tile([128, H], mybir.dt.float32)
                for r in range(R):
                    chunks = [int(a_pos[r]) + i for i in range(int(sched_a[r, t]))] + \
                             [na + int(b_pos[r]) + i for i in range(int(sched_b[r, t]))]
                    agg = pagg.tile([128, F], mybir.dt.float32)
                    for ci, ch in enumerate(chunks):
                        nc.tensor.matmul(
                            out=agg[:], lhsT=mk_t[:, ch * 128:(ch + 1) * 128],
                            rhs=msgs[:, ch, :],
                            start=(ci == 0), stop=(ci == len(chunks) - 1))
                    g = r * TILES + t
                    agg_s = ap_.tile([128, F], mybir.dt.bfloat16, tag='aggs')
                    nc.scalar.activation(
                        out=agg_s[:], in_=agg[:],
                        func=mybir.ActivationFunctionType.Copy,
                        scale=inv_sb[:, g:g + 1])
                    trp = ptr.tile([128, F], mybir.dt.bfloat16, tag='trp')
                    for c2 in range(FC):
                        nc.tensor.transpose(
                            out=trp[:, c2 * 128:(c2 + 1) * 128],
                            in_=agg_s[:, c2 * 128:(c2 + 1) * 128],
                            identity=id_sb[:])
                    aggT = atp.tile([128, F], mybir.dt.bfloat16, tag='aggT')
                    nc.vector.tensor_copy(out=aggT[:], in_=trp[:])
                    for c2 in range(FC):
                        nc.tensor.matmul(
                            out=opsum[:],
                            lhsT=aggT[:, c2 * 128:(c2 + 1) * 128],
                            rhs=W_sb[:, (r * FC + c2) * H:(r * FC + c2 + 1) * H],
                            start=(r == 0 and c2 == 0), stop=False)
                col_off += ct
                # root term + bias
                for c2 in range(FC):
                    nc.tensor.matmul(
                        out=opsum[:],
                        lhsT=xt_t[:, c2 * 128:(c2 + 1) * 128],
                        rhs=root_sb[:, c2 * H:(c2 + 1) * H],
                        start=False, stop=False)
                nc.tensor.matmul(
                    out=opsum[:], lhsT=ones_sb[:], rhs=b_sb[:],
                    start=False, stop=True)

                if layer == 1:
                    h_t = hp.tile([128, H], mybir.dt.bfloat16, tag='ht')
                    nc.scalar.activation(
                        out=h_t[:], in_=opsum[:],
                        func=mybir.ActivationFunctionType.Relu)
                    nc.sync.dma_start(
                        out=yout[t * 128:(t + 1) * 128, :], in_=h_t[:])
                else:
                    nrm2 = hp.tile([128, 1], mybir.dt.float32, tag='n2')
                    sq = hp.tile([128, OUT], mybir.dt.float32, tag='sq')
                    nc.scalar.activation(
                        out=sq[:], in_=opsum[:],
                        func=mybir.ActivationFunctionType.Square,
                        accum_out=nrm2[:])
                    srt = hp.tile([128, 1], mybir.dt.float32, tag='srt')
                    nc.scalar.activation(
                        out=srt[:], in_=nrm2[:],
                        func=mybir.ActivationFunctionType.Sqrt)
                    nc.vector.tensor_scalar_max(srt[:], srt[:], 1e-12)
                    rcp = hp.tile([128, 1], mybir.dt.float32, tag='rcp')
                    nc.vector.reciprocal(rcp[:], srt[:])
                    o_t = hp.tile([128, OUT], mybir.dt.float32, tag='ot')
                    nc.scalar.activation(
                        out=o_t[:], in_=opsum[:],
                        func=mybir.ActivationFunctionType.Copy,
                        scale=rcp[:])
                    nc.sync.dma_start(
                        out=yout[t * 128:(t + 1) * 128, :], in_=o_t[:])

    import concourse.mybir as mybir2
    mybir2.codegen_inst_isa_subclasses(nc)
    return nc


def _run(nc, in_maps, trace=False):
    from concourse import bass_utils
    res = bass_utils.run_bass_kernel_spmd(
        nc, in_maps, core_ids=list(range(NCORES)), trace=trace)
    return res


# ---------------------------------------------------------------------------
# Entry point
# ---------------------------------------------------------------------------
def kernel(x, W1, root1, b1, W2, root2, b2, src, dst, edge_type,
           _trace=None):
    _install_tilefix()
    _install_ntff_hook()

    src = np.asarray(src).astype(np.int64)
    dst = np.asarray(dst).astype(np.int64)
    et = np.asarray(edge_type).astype(np.int64)
    x = np.asarray(x, np.float32)

    pad_src = (src // SHARD) * PSH + (src % SHARD)   # row in padded h layout
    sched_a, sched_b, per_core = _host_prep(src, dst, et, pad_src)

    ident_np = np.eye(128, dtype=np.float32).astype(bf16)

    # stage x into the same padded [core][tile][row] layout as h
    x_pad = np.zeros((NCORES * PSH, IN), bf16)
    for c in range(NCORES):
        x_pad[c * PSH:c * PSH + SHARD] = x[c * SHARD:(c + 1) * SHARD].astype(bf16)
    W1p = _pack_weights(np.asarray(W1, np.float32), IN // 128)
    r1p = _pack_single(np.asarray(root1, np.float32), IN // 128)
    b1p = np.asarray(b1, np.float32)[None, :].astype(bf16)
    W2p = _pack_weights(np.asarray(W2, np.float32), HID // 128)
    r2p = _pack_single(np.asarray(root2, np.float32), HID // 128)
    b2p = np.asarray(b2, np.float32)[None, :].astype(bf16)

    # ---- layer 1: single launch ----
    nc1 = _build_layer(1, sched_a, sched_b)
    in_maps1 = []
    for c in range(NCORES):
        pc = per_core[c]
        in_maps1.append(dict(
            xsrc=x_pad, xT=_shard_T(x, c, IN), Wsb=W1p, rootsb=r1p,
            brow=b1p, idxa=pc['idxa'], idxb=pc['idxb'],
            masks=pc['masks'], invc=pc['invc'], ident=ident_np))
    res1 = _run(nc1, in_maps1, trace=(_trace == 'l1_0'))
    _pending_trace['l1'] = res1.exec_time_ns
    h_full = np.zeros((NCORES * PSH, HID), bf16)
    for c in range(NCORES):
        h_full[c * PSH:(c + 1) * PSH] = res1.results[c]['yout']

    # ---- layer 2: single launch ----
    h_f32 = h_full.astype(np.float32)
    nc2 = _build_layer(2, sched_a, sched_b)
    in_maps2 = []
    for c in range(NCORES):
        pc = per_core[c]
        in_maps2.append(dict(
            xsrc=h_full, xT=_shard_T(h_f32, c, HID, PSH), Wsb=W2p,
            rootsb=r2p, brow=b2p, idxa=pc['idxa'], idxb=pc['idxb'],
            masks=pc['masks'], invc=pc['invc'], ident=ident_np))
    res2 = _run(nc2, in_maps2, trace=(_trace == 'l2_0'))
    _pending_trace['l2'] = res2.exec_time_ns
    out = np.empty((N, OUT), np.float32)
    for c in range(NCORES):
        out[c * SHARD:(c + 1) * SHARD] = \
            res2.results[c]['yout'][:SHARD].astype(np.float32)
    return out
